# revision 81
# baseline (speedup 1.0000x reference)
"""Multi-head self-attention Trainium2 kernel (B=4, T=2048, C=1024, H=16, D=64).

Sharding: 8 cores = 4 batches x 2 head-groups (8 heads each). Each core
computes its batch's QKV for its heads, attention, and a partial output
projection (row-sharded over attention features). The host sums the two
partials per batch and adds b_proj + bv @ w_proj.T (the V bias times the
softmax weights, which sum to 1).

Optimizations over the straightforward version:
  - Scores run as fp8e4 DoubleRow matmuls (0.5 cycles/row): Q/K are
    quantized to fp8 by the bias-add, stored [128, 2, 2048] with the
    second k-subtile zeroed (D=64 < 128, so the pair is (dims, zeros)).
  - P@V runs transposed: out[tok, dim] with P as stationary
    ([128 keys, 128 tok] slices) and V[keys, 65] as moving (ones column
    produces the softmax denominator), so each matmul moves 65 elements
    instead of 512.
  - Softmax normalization becomes a per-partition scalar multiply, then
    PE transposes restore [feat, tok] tiles for the output projection.
  - exp is split across ScalarE (native exp) and DVE (bitwise fast-exp
    into bf16 bits); GPSIMD cannot read PSUM so only these two drain
    score tiles.
  - The whole schedule is one software pipeline: each block's score
    matmuls interleave with the previous block's P@V accumulation and
    with projection / output-projection pieces, so the PE never waits
    long on the 4-slot score-PSUM ring.
"""

import numpy as np
import ml_dtypes
from contextlib import ExitStack

import concourse.bass as bass
import concourse.bacc as bacc
import concourse.tile as tile
import concourse.mybir as mybir
from concourse.bass_utils import run_bass_kernel_spmd

F32 = mybir.dt.float32
BF16 = mybir.dt.bfloat16
FP8 = mybir.dt.float8e4
I16 = mybir.dt.int16
BF16_NP = ml_dtypes.bfloat16

B, T, C = 4, 2048, 1024
H, D = 16, 64
HL = 8          # heads per core
N_CORES = 8
CC = C // 128   # 8 contraction chunks for QKV
TB = T // 512   # 4 token blocks of 512
TT = T // 128   # 16 token chunks of 128
EXPFN = mybir.ActivationFunctionType.Exp
DR = mybir.MatmulPerfMode.DoubleRow

USE_FP8_QK = True

# Schraudolph-style exp in bf16 bit space: i16 = x*SCHR_A + SCHR_B, then
# reinterpret the int16 as bf16.  SCHR_A folds the 1/sqrt(D) score scale.
SCHR_A = 0.125 * 128.0 / float(np.log(2.0))
SCHR_B = 127.0 * 128.0 - 7.41

# exp engine per (kp, j2, kc-half) slot (32 tiles of [128,512] per
# block): 'A' = ScalarE native exp, 'V' = DVE bitwise fast-exp.  GPSIMD
# cannot read PSUM, so only these two engines can consume score tiles.
# Pairs per (kp, j2) stay on one engine so the 4-deep PSUM ring
# alternates engines every two slots; 18 A / 14 V balances the two
# engines' total load (DVE also carries reciprocals, normalize and
# copies).
EXP_ENG = (['A', 'A', 'V', 'V'] * 6 +
           ['A', 'A', 'A', 'V'] * 2)


def build_program():
    nc = bacc.Bacc("TRN2", debug=False, num_devices=1, target_bir_lowering=False)

    xT = nc.dram_tensor("xT", [C, T], BF16, kind="ExternalInput").ap()
    wqT = nc.dram_tensor("wqT", [C, 512], BF16, kind="ExternalInput").ap()
    wkT = nc.dram_tensor("wkT", [C, 512], BF16, kind="ExternalInput").ap()
    wvT = nc.dram_tensor("wvT", [C, 512], BF16, kind="ExternalInput").ap()
    bqk = nc.dram_tensor("bqk", [128, 8], F32, kind="ExternalInput").ap()
    wpT = nc.dram_tensor("wpT", [4, 128, 1024], BF16, kind="ExternalInput").ap()
    ident = nc.dram_tensor("ident", [128, 128], BF16, kind="ExternalInput").ap()
    yp = nc.dram_tensor("yp", [T, C], F32, kind="ExternalOutput").ap()

    qk_dt = FP8 if USE_FP8_QK else BF16

    with tile.TileContext(nc) as tc, ExitStack() as top:
        cpool = top.enter_context(tc.tile_pool(name="consts", bufs=1))
        bqk_sb = cpool.tile([128, 8], F32, tag="bqk")
        id_sb = cpool.tile([128, 128], BF16, tag="ident")

        actpool = top.enter_context(tc.tile_pool(name="acts", bufs=1))
        # OT: attention output, feature-major [feat 128, tok 512] bf16
        OT = {(g, qb): actpool.tile([128, 512], BF16, tag=f"ot{g}_{qb}",
                                    name=f"ot{g}_{qb}")
              for g in range(4) for qb in range(4)}
        # Q/K in DoubleRow layout: [128 feat(2 heads), 2 k-subtiles, 2048 tok]
        # subtile 1 is zeros (fp8) / unused (bf16).
        QDR = {g: actpool.tile([128, 2, T], qk_dt, tag=f"qdr{g}",
                               name=f"qdr{g}") for g in range(4)}
        KDR = {g: actpool.tile([128, 2, T], qk_dt, tag=f"kdr{g}",
                               name=f"kdr{g}") for g in range(4)}
        V = [actpool.tile([128, HL * 65], BF16, tag=f"v{tt}", name=f"v{tt}")
             for tt in range(TT)]

        if USE_FP8_QK:
            # zero the second k-subtile once (matmul contracts over both)
            for g in range(4):
                nc.gpsimd.memset(QDR[g][:, 1, :], 0.0)
                nc.gpsimd.memset(KDR[g][:, 1, :], 0.0)

        ps1cm = tc.tile_pool(name="ps1", bufs=2, space="PSUM")
        ps1pool = ps1cm.__enter__()
        attncm = [tc.tile_pool(name="pt", bufs=1),
                  tc.tile_pool(name="ps2", bufs=4, space="PSUM"),
                  tc.tile_pool(name="pvtr", bufs=2, space="PSUM"),
                  tc.tile_pool(name="rr", bufs=2),
                  tc.tile_pool(name="otm", bufs=2)]
        (ptpool, ps2pool, pvpool, rrpool, otmpool) = \
            [cm.__enter__() for cm in attncm]

        xbcm = tc.tile_pool(name="xball", bufs=1)
        xbpool = xbcm.__enter__()

        # x feature-major, all of it: [128, cc, tok]; loaded in 4 big DMAs
        xar = xbpool.tile([128, CC, T], BF16, tag="xar")
        xTr = xT.rearrange("(c p) t -> p c t", p=128)
        wq_all = xbpool.tile([128, CC * 512], BF16, tag="wq_all")
        wqr = wqT.rearrange("(c p) j -> p c j", p=128)
        wqv = wq_all[:].rearrange("p (c j) -> p c j", j=512)
        wk_all = xbpool.tile([128, CC * 512], BF16, tag="wk_all")
        nc.sync.dma_start(wqv[:, 0:4], wqr[:, 0:4])
        nc.sync.dma_start(xar[:, :, 0:512], xTr[:, :, 0:512])
        nc.sync.dma_start(wqv[:, 4:8], wqr[:, 4:8])
        nc.sync.dma_start(bqk_sb[:], bqk[:])
        nc.sync.dma_start(
            wk_all[:].rearrange("p (c j) -> p c j", j=512),
            wkT.rearrange("(c p) j -> p c j", p=128))
        for tb in range(1, TB):
            nc.sync.dma_start(xar[:, :, tb * 512:(tb + 1) * 512],
                              xTr[:, :, tb * 512:(tb + 1) * 512])
        wv_all = xbpool.tile([128, CC, 512], BF16, tag="wv_all")
        nc.sync.dma_start(wv_all[:],
                          wvT.rearrange("(c p) j -> p c j", p=128))
        nc.sync.dma_start(id_sb[:], ident[:])

        def xb(tb, cc):
            return xar[:, cc, tb * 512:(tb + 1) * 512]

        def qk_proj_piece(g, tb, which):
            """One Q or K projection group for head pair g, token block tb.
            The bias add runs on ScalarE (per-partition bias AP is legal
            there) and quantizes to fp8 on the way out."""
            w_all, dst, bcol = ((wq_all, QDR[g], g) if which == 0 else
                                (wk_all, KDR[g], 4 + g))
            ps = ps1pool.tile([128, 512], F32, tag="ps1", name="psqk")
            for cc in range(CC):
                co = cc * 512 + g * 128
                nc.tensor.matmul(
                    ps[:], w_all[:, co:co + 128],
                    xb(tb, cc),
                    start=(cc == 0), stop=(cc == CC - 1))
            nc.scalar.activation(
                dst[:, 0, tb * 512:(tb + 1) * 512], ps[:],
                mybir.ActivationFunctionType.Identity,
                bias=bqk_sb[:, bcol:bcol + 1])

        def qk_proj(g):
            for tb in range(TB):
                for which in range(2):
                    qk_proj_piece(g, tb, which)

        def v_proj_piece(tt):
            ps = ps1pool.tile([128, 512], F32, tag="ps1", name="psv")
            for cc in range(CC):
                nc.tensor.matmul(
                    ps[:], xar[:, cc, tt * 128:(tt + 1) * 128],
                    wv_all[:, cc, :],
                    start=(cc == 0), stop=(cc == CC - 1))
            # V carries no bias: softmax weights sum to 1, so the bias
            # contribution is bv @ Wp per token, folded into the host-side
            # b_proj add.
            v3 = V[tt][:].rearrange("p (h x) -> p h x", x=65)
            nc.gpsimd.memset(v3[:, :, 64:65], 1.0)
            src = ps[:].rearrange("p (h x) -> p h x", x=64)
            nc.vector.tensor_copy(v3[:, :, 0:64], src)

        # ---- attention blocks, software-pipelined -----------------------
        # block n = (g, qb), qb-major so every fourth block finishes an
        # OT column and the output projection can interleave early.
        # scores(n) and PV(n-1) interleave in the PE stream so PE has
        # work while exp drains score PSUMs.
        blocks = [(g, qb) for qb in range(4) for g in range(4)]
        pts = {}      # (parity, kp, j2) -> exp'd score tile [128, 1024]
        pv_state = {}  # live PV psum tiles per j2

        def scores_mm(n, g, qb, kp, j2):
            """Scores for head 2g+j2, key chunks 2kp/2kp+1, queries qb:
            two [128, 512] tiles (one per key chunk), each exp'd as soon
            as it fills."""
            fo = 64 * j2
            for j in range(2):
                kc = 2 * kp + j
                pp = ps2pool.tile([128, 512], F32, tag="ps2",
                                  name=f"sc{j2}")
                if USE_FP8_QK:
                    for u in range(2):
                        nc.tensor.matmul(
                            pp[:, u * 256:(u + 1) * 256],
                            KDR[g][fo:fo + 64, :, kc * 128:(kc + 1) * 128],
                            QDR[g][fo:fo + 64, :,
                                   qb * 512 + u * 256:qb * 512 + (u + 1) * 256],
                            start=True, stop=True, perf_mode=DR)
                else:
                    nc.tensor.matmul(
                        pp[:],
                        KDR[g][fo:fo + 64, 0, kc * 128:(kc + 1) * 128],
                        QDR[g][fo:fo + 64, 0, qb * 512:(qb + 1) * 512],
                        start=True, stop=True)
                pt = ptpool.tile([128, 512], BF16,
                                 tag=f"pt{n % 2}_{kc}_{j2}",
                                 name=f"pt{n % 2}_{kc}_{j2}")
                if EXP_ENG[4 * kp + 2 * j2 + j] == 'A':
                    nc.scalar.activation(pt[:], pp[:], EXPFN, scale=0.125)
                else:
                    nc.vector.tensor_scalar(pt[:].bitcast(I16), pp[:],
                                            SCHR_A, SCHR_B,
                                            op0=mybir.AluOpType.mult,
                                            op1=mybir.AluOpType.add)
                pts[(n % 2, kc, j2)] = pt

        # PV accumulation order per head half: groups ts0..ts3, 16 key
        # chunks each, strictly sequential (one open accumulation group
        # per PSUM bank).  Spread over steps 0..5 so the finish work can
        # run at steps 6-7 without delaying the next block's exps.
        PV_SPLIT = [0, 11, 22, 33, 44, 54, 64]

        def pv_mm(n, g, qb, step):
            par = n % 2
            for j2 in range(2):
                h = 2 * g + j2
                if step == 0:
                    pv_state[j2] = pvpool.tile([128, 260], F32,
                                               tag="pvtr", name=f"pv{j2}")
                pv = pv_state[j2]
                for i in range(PV_SPLIT[step], PV_SPLIT[step + 1]):
                    ts, kc = i // 16, i % 16
                    nc.tensor.matmul(
                        pv[:, ts * 65:(ts + 1) * 65],
                        pts[(par, kc, j2)][:, ts * 128:(ts + 1) * 128],
                        V[kc][:, h * 65:(h + 1) * 65],
                        start=(kc == 0), stop=(kc == 15))

        otm_state = {}

        def pv_finish_a(n, g, qb):
            """Normalize block n's PV accumulators (runs at step 6):
            reciprocal on DVE, the per-token scale on ScalarE."""
            for j2 in range(2):
                pv = pv_state.pop(j2)
                rr = rrpool.tile([128, 4], F32, tag="rr", name="rr")
                pv3 = pv[:].rearrange("p (t x) -> p t x", x=65)
                nc.vector.reciprocal(rr[:], pv3[:, :, 64])
                ot = otmpool.tile([128, 256], BF16, tag=f"otm{j2}",
                                  name=f"otm{j2}")
                nc.vector.tensor_tensor(
                    ot[:].rearrange("p (t x) -> p t x", x=64),
                    pv3[:, :, 0:64],
                    rr[:].unsqueeze(-1).broadcast_to([128, 4, 64]),
                    op=mybir.AluOpType.mult)
                otm_state[j2] = ot

        def pv_finish_b(n, g, qb):
            """Transpose + store OT tiles for block n (runs at step 7)."""
            otm = {j2: otm_state.pop(j2) for j2 in range(2)}
            for ts in range(4):
                tr = ps1pool.tile([128, 128], BF16, tag="ps1", name="tr")
                for j2 in range(2):
                    nc.tensor.matmul(
                        tr[64 * j2:64 * j2 + 64, :],
                        otm[j2][:, ts * 64:(ts + 1) * 64],
                        id_sb[:], start=True, stop=True, is_transpose=True)
                nc.vector.tensor_copy(
                    OT[(g, qb)][:, ts * 128:(ts + 1) * 128], tr[:])

        # ---- output projection piece (interleaved into late blocks) ----
        wp_state = {}

        def outproj_load():
            wpcm = tc.tile_pool(name="wp", bufs=1)
            ycm = tc.tile_pool(name="ysb", bufs=2)
            wp_state["cms"] = [wpcm, ycm]
            wppool = wpcm.__enter__()
            ypool = ycm.__enter__()
            wp_all = wppool.tile([128, 4096], BF16, tag="wp_all")
            nc.sync.dma_start(
                wp_all[:].rearrange("p (j o) -> p j o", o=1024),
                wpT.rearrange("j p o -> p j o"))
            wp_state.update(wp_all=wp_all, ypool=ypool)

        def outproj_piece(tt):
            # b_proj is added on the host; this is a plain PSUM drain,
            # split across ScalarE and DVE.
            wp_all = wp_state["wp_all"]
            y_sb = wp_state["ypool"].tile([128, 1024], F32, tag="y",
                                          name="y_sb")
            for cb in range(2):
                ps = ps1pool.tile([128, 512], F32, tag="ps1", name="psy")
                for j in range(4):
                    nc.tensor.matmul(
                        ps[:],
                        OT[(j, tt // 4)][:, (tt % 4) * 128:
                                         (tt % 4 + 1) * 128],
                        wp_all[:, j * 1024 + cb * 512:
                               j * 1024 + (cb + 1) * 512],
                        start=(j == 0), stop=(j == 3))
                if cb == 0:
                    nc.scalar.copy(y_sb[:, 0:512], ps[:])
                else:
                    nc.vector.tensor_copy(y_sb[:, 512:1024], ps[:])
                nc.sync.dma_start(
                    yp[tt * 128:(tt + 1) * 128, cb * 512:(cb + 1) * 512],
                    y_sb[:, cb * 512:(cb + 1) * 512])

        # ---- emit ------------------------------------------------------
        # outproj tile tt is ready once OT[(3, tt//4)] exists, i.e. after
        # pv_finish_b(block 4*(tt//4)+3) which is emitted during block
        # 4*(tt//4)+4; schedule one tile per block starting one later so
        # PE-light blocks all carry some slack work.
        outmap = {5: [0], 6: [1], 7: [2], 8: [3], 9: [4], 10: [5],
                  11: [6], 12: [7], 13: [8], 14: [9], 15: [10, 11]}
        # startup: only the q/k tiles the first scores need; the rest of
        # projection 0, V, and projection 1 interleave into block 0 in
        # deadline order (K chunk tb feeds scores step 2*tb; V feeds
        # block 1's PV; projection 1 feeds block 1).
        qk_proj_piece(0, 0, 0)
        qk_proj_piece(0, 0, 1)
        for n, (g, qb) in enumerate(blocks):
            if n == 4:
                # x / qkv-weight tiles are dead; reuse their SBUF for the
                # output projection weights
                xbcm.__exit__(None, None, None)
                outproj_load()
            extra = []
            if n == 0:
                extra = [(qk_proj_piece, (0, tb, 1)) for tb in (1, 2, 3)]
                extra += [(v_proj_piece, (tt,)) for tt in range(8)]
                extra += [(qk_proj_piece, (1, 0, w)) for w in range(2)]
                extra += [(v_proj_piece, (tt,)) for tt in range(8, TT)]
                extra += [(qk_proj_piece, (1, tb, w))
                          for tb in (1, 2, 3) for w in range(2)]
                extra += [(qk_proj_piece, (0, tb, 0)) for tb in (1, 2, 3)]
            elif n < 3:
                extra = [(qk_proj_piece, (n + 1, tb, w))
                         for tb in range(TB) for w in range(2)]
            extra += [(outproj_piece, (tt,)) for tt in outmap.get(n, [])]
            npc = (len(extra) + 7) // 8  # extra pieces per step
            for kp in range(8):
                if n > 0:
                    if kp < 6:
                        pv_mm(n - 1, *blocks[n - 1], step=kp)
                    elif kp == 6:
                        pv_finish_a(n - 1, *blocks[n - 1])
                    else:
                        pv_finish_b(n - 1, *blocks[n - 1])
                if n == 0:
                    scores_mm(n, g, qb, kp, 0)
                    scores_mm(n, g, qb, kp, 1)
                for fn, args in extra[kp * npc:(kp + 1) * npc]:
                    fn(*args)
                if n > 0:
                    scores_mm(n, g, qb, kp, 0)
                    scores_mm(n, g, qb, kp, 1)
        for kp in range(6):
            pv_mm(15, *blocks[15], step=kp)
        pv_finish_a(15, *blocks[15])
        pv_finish_b(15, *blocks[15])
        for tt in range(12, 16):
            outproj_piece(tt)

        for cm in reversed(wp_state["cms"]):
            cm.__exit__(None, None, None)
        for cm in reversed(attncm):
            cm.__exit__(None, None, None)
        ps1cm.__exit__(None, None, None)

    nc.compile()
    return nc


_NC_CACHE = None


def get_program():
    global _NC_CACHE
    if _NC_CACHE is None:
        _NC_CACHE = build_program()
    return _NC_CACHE


def make_in_maps(x, w_qkv, b_qkv, w_proj, b_proj):
    x = np.asarray(x, dtype=np.float32)
    w_qkv = np.asarray(w_qkv, dtype=np.float32)
    b_qkv = np.asarray(b_qkv, dtype=np.float32)
    w_proj = np.asarray(w_proj, dtype=np.float32)
    b_proj = np.asarray(b_proj, dtype=np.float32)

    xTs = [np.ascontiguousarray(x[b].T).astype(BF16_NP) for b in range(B)]
    ident = np.eye(128, dtype=np.float32).astype(BF16_NP)

    grp = []
    for hg in range(2):
        sl = slice(hg * 512, (hg + 1) * 512)
        wq = w_qkv[0:C][sl]
        wk = w_qkv[C:2 * C][sl]
        wv = w_qkv[2 * C:3 * C][sl]
        bq = b_qkv[0:C][sl]
        bk = b_qkv[C:2 * C][sl]
        bv = b_qkv[2 * C:3 * C][sl]
        grp.append(dict(
            wqT=np.ascontiguousarray(wq.T).astype(BF16_NP),
            wkT=np.ascontiguousarray(wk.T).astype(BF16_NP),
            wvT=np.ascontiguousarray(wv.T).astype(BF16_NP),
            bqk=np.stack([bq[i * 128:(i + 1) * 128] for i in range(4)]
                         + [bk[i * 128:(i + 1) * 128] for i in range(4)],
                         axis=1).astype(np.float32),
            wpT=np.ascontiguousarray(
                w_proj[:, sl].T).reshape(4, 128, 1024).astype(BF16_NP),
            ident=ident,
        ))

    in_maps = []
    for core in range(N_CORES):
        b, hg = core // 2, core % 2
        m = {"xT": xTs[b]}
        m.update(grp[hg])
        in_maps.append(m)
    return in_maps


def kernel(x, w_qkv, b_qkv, w_proj, b_proj):
    nc = get_program()
    in_maps = make_in_maps(x, w_qkv, b_qkv, w_proj, b_proj)
    res = run_bass_kernel_spmd(
        nc, in_maps, core_ids=list(range(N_CORES)), trace=False)
    # V's bias is not applied on-device: softmax weights sum to 1, so it
    # contributes bv @ w_proj.T per token, folded in here with b_proj.
    bp = (np.asarray(b_proj, dtype=np.float32)
          + np.asarray(b_qkv, dtype=np.float32)[2 * C:]
          @ np.asarray(w_proj, dtype=np.float32).T)
    y = np.empty((B, T, C), dtype=np.float32)
    for b in range(B):
        y[b] = res.results[2 * b]["yp"] + res.results[2 * b + 1]["yp"] + bp
    return y


# revision 86
# speedup vs baseline: 1.0004x; 1.0004x over previous
"""Multi-head self-attention Trainium2 kernel (B=4, T=2048, C=1024, H=16, D=64).

Sharding: 8 cores = 4 batches x 2 head-groups (8 heads each). Each core
computes its batch's QKV for its heads, attention, and a partial output
projection (row-sharded over attention features). The host sums the two
partials per batch and adds b_proj + bv @ w_proj.T (the V bias times the
softmax weights, which sum to 1).

Optimizations over the straightforward version:
  - Scores run as fp8e4 DoubleRow matmuls (0.5 cycles/row): Q/K are
    quantized to fp8 by the bias-add, stored [128, 2, 2048] with the
    second k-subtile zeroed (D=64 < 128, so the pair is (dims, zeros)).
  - P@V runs transposed: out[tok, dim] with P as stationary
    ([128 keys, 128 tok] slices) and V[keys, 65] as moving (ones column
    produces the softmax denominator), so each matmul moves 65 elements
    instead of 512.
  - Softmax normalization becomes a per-partition scalar multiply, then
    PE transposes restore [feat, tok] tiles for the output projection.
  - exp is split across ScalarE (native exp) and DVE (bitwise fast-exp
    into bf16 bits); GPSIMD cannot read PSUM so only these two drain
    score tiles.
  - The whole schedule is one software pipeline: each block's score
    matmuls interleave with the previous block's P@V accumulation and
    with projection / output-projection pieces, so the PE never waits
    long on the 4-slot score-PSUM ring.
"""

import numpy as np
import ml_dtypes
from contextlib import ExitStack

import concourse.bass as bass
import concourse.bacc as bacc
import concourse.tile as tile
import concourse.mybir as mybir
from concourse.bass_utils import run_bass_kernel_spmd

F32 = mybir.dt.float32
BF16 = mybir.dt.bfloat16
FP8 = mybir.dt.float8e4
I16 = mybir.dt.int16
BF16_NP = ml_dtypes.bfloat16

B, T, C = 4, 2048, 1024
H, D = 16, 64
HL = 8          # heads per core
N_CORES = 8
CC = C // 128   # 8 contraction chunks for QKV
TB = T // 512   # 4 token blocks of 512
TT = T // 128   # 16 token chunks of 128
EXPFN = mybir.ActivationFunctionType.Exp
DR = mybir.MatmulPerfMode.DoubleRow

USE_FP8_QK = True

# Schraudolph-style exp in bf16 bit space: i16 = x*SCHR_A + SCHR_B, then
# reinterpret the int16 as bf16.  SCHR_A folds the 1/sqrt(D) score scale.
SCHR_A = 0.125 * 128.0 / float(np.log(2.0))
SCHR_B = 127.0 * 128.0 - 7.41

# exp engine per (kp, j2, kc-half) slot (32 tiles of [128,512] per
# block): 'A' = ScalarE native exp, 'V' = DVE bitwise fast-exp.  GPSIMD
# cannot read PSUM, so only these two engines can consume score tiles.
# Pairs per (kp, j2) stay on one engine so the 4-deep PSUM ring
# alternates engines every two slots; 18 A / 14 V balances the two
# engines' total load (DVE also carries reciprocals, normalize and
# copies).
EXP_ENG = ['A', 'V'] * 14 + ['A'] * 4


def build_program():
    nc = bacc.Bacc("TRN2", debug=False, num_devices=1, target_bir_lowering=False)

    xT = nc.dram_tensor("xT", [C, T], BF16, kind="ExternalInput").ap()
    wqT = nc.dram_tensor("wqT", [C, 512], BF16, kind="ExternalInput").ap()
    wkT = nc.dram_tensor("wkT", [C, 512], BF16, kind="ExternalInput").ap()
    wvT = nc.dram_tensor("wvT", [C, 512], BF16, kind="ExternalInput").ap()
    bqk = nc.dram_tensor("bqk", [128, 8], F32, kind="ExternalInput").ap()
    wpT = nc.dram_tensor("wpT", [4, 128, 1024], BF16, kind="ExternalInput").ap()
    ident = nc.dram_tensor("ident", [128, 128], BF16, kind="ExternalInput").ap()
    yp = nc.dram_tensor("yp", [T, C], F32, kind="ExternalOutput").ap()

    qk_dt = FP8 if USE_FP8_QK else BF16

    with tile.TileContext(nc) as tc, ExitStack() as top:
        cpool = top.enter_context(tc.tile_pool(name="consts", bufs=1))
        bqk_sb = cpool.tile([128, 8], F32, tag="bqk")
        id_sb = cpool.tile([128, 128], BF16, tag="ident")

        actpool = top.enter_context(tc.tile_pool(name="acts", bufs=1))
        # OT: attention output, feature-major [feat 128, tok 512] bf16
        OT = {(g, qb): actpool.tile([128, 512], BF16, tag=f"ot{g}_{qb}",
                                    name=f"ot{g}_{qb}")
              for g in range(4) for qb in range(4)}
        # Q/K in DoubleRow layout: [128 feat(2 heads), 2 k-subtiles, 2048 tok]
        # subtile 1 is zeros (fp8) / unused (bf16).
        QDR = {g: actpool.tile([128, 2, T], qk_dt, tag=f"qdr{g}",
                               name=f"qdr{g}") for g in range(4)}
        KDR = {g: actpool.tile([128, 2, T], qk_dt, tag=f"kdr{g}",
                               name=f"kdr{g}") for g in range(4)}
        V = [actpool.tile([128, HL * 65], BF16, tag=f"v{tt}", name=f"v{tt}")
             for tt in range(TT)]

        if USE_FP8_QK:
            # zero the second k-subtile once (matmul contracts over both)
            for g in range(4):
                nc.gpsimd.memset(QDR[g][:, 1, :], 0.0)
                nc.gpsimd.memset(KDR[g][:, 1, :], 0.0)

        ps1cm = tc.tile_pool(name="ps1", bufs=2, space="PSUM")
        ps1pool = ps1cm.__enter__()
        attncm = [tc.tile_pool(name="pt", bufs=1),
                  tc.tile_pool(name="ps2", bufs=4, space="PSUM"),
                  tc.tile_pool(name="pvtr", bufs=2, space="PSUM"),
                  tc.tile_pool(name="rr", bufs=2),
                  tc.tile_pool(name="otm", bufs=2)]
        (ptpool, ps2pool, pvpool, rrpool, otmpool) = \
            [cm.__enter__() for cm in attncm]

        xbcm = tc.tile_pool(name="xball", bufs=1)
        xbpool = xbcm.__enter__()

        # x feature-major, all of it: [128, cc, tok]; loaded in 4 big DMAs
        xar = xbpool.tile([128, CC, T], BF16, tag="xar")
        xTr = xT.rearrange("(c p) t -> p c t", p=128)
        wq_all = xbpool.tile([128, CC * 512], BF16, tag="wq_all")
        wqr = wqT.rearrange("(c p) j -> p c j", p=128)
        wqv = wq_all[:].rearrange("p (c j) -> p c j", j=512)
        wk_all = xbpool.tile([128, CC * 512], BF16, tag="wk_all")
        nc.sync.dma_start(wqv[:, 0:4], wqr[:, 0:4])
        nc.sync.dma_start(xar[:, :, 0:512], xTr[:, :, 0:512])
        nc.sync.dma_start(wqv[:, 4:8], wqr[:, 4:8])
        nc.sync.dma_start(bqk_sb[:], bqk[:])
        nc.sync.dma_start(
            wk_all[:].rearrange("p (c j) -> p c j", j=512),
            wkT.rearrange("(c p) j -> p c j", p=128))
        for tb in range(1, TB):
            nc.sync.dma_start(xar[:, :, tb * 512:(tb + 1) * 512],
                              xTr[:, :, tb * 512:(tb + 1) * 512])
        wv_all = xbpool.tile([128, CC, 512], BF16, tag="wv_all")
        nc.sync.dma_start(wv_all[:],
                          wvT.rearrange("(c p) j -> p c j", p=128))
        nc.sync.dma_start(id_sb[:], ident[:])

        def xb(tb, cc):
            return xar[:, cc, tb * 512:(tb + 1) * 512]

        def qk_proj_piece(g, tb, which):
            """One Q or K projection group for head pair g, token block tb.
            The bias add runs on ScalarE (per-partition bias AP is legal
            there) and quantizes to fp8 on the way out."""
            w_all, dst, bcol = ((wq_all, QDR[g], g) if which == 0 else
                                (wk_all, KDR[g], 4 + g))
            ps = ps1pool.tile([128, 512], F32, tag="ps1", name="psqk")
            for cc in range(CC):
                co = cc * 512 + g * 128
                nc.tensor.matmul(
                    ps[:], w_all[:, co:co + 128],
                    xb(tb, cc),
                    start=(cc == 0), stop=(cc == CC - 1))
            nc.scalar.activation(
                dst[:, 0, tb * 512:(tb + 1) * 512], ps[:],
                mybir.ActivationFunctionType.Identity,
                bias=bqk_sb[:, bcol:bcol + 1])

        def qk_proj(g):
            for tb in range(TB):
                for which in range(2):
                    qk_proj_piece(g, tb, which)

        def v_proj_piece(tt):
            ps = ps1pool.tile([128, 512], F32, tag="ps1", name="psv")
            for cc in range(CC):
                nc.tensor.matmul(
                    ps[:], xar[:, cc, tt * 128:(tt + 1) * 128],
                    wv_all[:, cc, :],
                    start=(cc == 0), stop=(cc == CC - 1))
            # V carries no bias: softmax weights sum to 1, so the bias
            # contribution is bv @ Wp per token, folded into the host-side
            # b_proj add.
            v3 = V[tt][:].rearrange("p (h x) -> p h x", x=65)
            nc.gpsimd.memset(v3[:, :, 64:65], 1.0)
            src = ps[:].rearrange("p (h x) -> p h x", x=64)
            nc.vector.tensor_copy(v3[:, :, 0:64], src)

        # ---- attention blocks, software-pipelined -----------------------
        # block n = (g, qb), qb-major so every fourth block finishes an
        # OT column and the output projection can interleave early.
        # scores(n) and PV(n-1) interleave in the PE stream so PE has
        # work while exp drains score PSUMs.
        blocks = [(g, qb) for qb in range(4) for g in range(4)]
        pts = {}      # (parity, kp, j2) -> exp'd score tile [128, 1024]
        pv_state = {}  # live PV psum tiles per j2

        def scores_mm(n, g, qb, kp, j2):
            """Scores for head 2g+j2, key chunks 2kp/2kp+1, queries qb:
            two [128, 512] tiles (one per key chunk), each exp'd as soon
            as it fills."""
            fo = 64 * j2
            for j in range(2):
                kc = 2 * kp + j
                pp = ps2pool.tile([128, 512], F32, tag="ps2",
                                  name=f"sc{j2}")
                if USE_FP8_QK:
                    for u in range(2):
                        nc.tensor.matmul(
                            pp[:, u * 256:(u + 1) * 256],
                            KDR[g][fo:fo + 64, :, kc * 128:(kc + 1) * 128],
                            QDR[g][fo:fo + 64, :,
                                   qb * 512 + u * 256:qb * 512 + (u + 1) * 256],
                            start=True, stop=True, perf_mode=DR)
                else:
                    nc.tensor.matmul(
                        pp[:],
                        KDR[g][fo:fo + 64, 0, kc * 128:(kc + 1) * 128],
                        QDR[g][fo:fo + 64, 0, qb * 512:(qb + 1) * 512],
                        start=True, stop=True)
                pt = ptpool.tile([128, 512], BF16,
                                 tag=f"pt{n % 2}_{kc}_{j2}",
                                 name=f"pt{n % 2}_{kc}_{j2}")
                if EXP_ENG[4 * kp + 2 * j2 + j] == 'A':
                    nc.scalar.activation(pt[:], pp[:], EXPFN, scale=0.125)
                else:
                    nc.vector.tensor_scalar(pt[:].bitcast(I16), pp[:],
                                            SCHR_A, SCHR_B,
                                            op0=mybir.AluOpType.mult,
                                            op1=mybir.AluOpType.add)
                pts[(n % 2, kc, j2)] = pt

        # PV accumulation order per head half: groups ts0..ts3, 16 key
        # chunks each, strictly sequential (one open accumulation group
        # per PSUM bank).  Spread over steps 0..5 so the finish work can
        # run at steps 6-7 without delaying the next block's exps.
        PV_SPLIT = [0, 11, 22, 33, 44, 54, 64]

        def pv_mm(n, g, qb, step):
            par = n % 2
            for j2 in range(2):
                h = 2 * g + j2
                if step == 0:
                    pv_state[j2] = pvpool.tile([128, 260], F32,
                                               tag="pvtr", name=f"pv{j2}")
                pv = pv_state[j2]
                for i in range(PV_SPLIT[step], PV_SPLIT[step + 1]):
                    ts, kc = i // 16, i % 16
                    nc.tensor.matmul(
                        pv[:, ts * 65:(ts + 1) * 65],
                        pts[(par, kc, j2)][:, ts * 128:(ts + 1) * 128],
                        V[kc][:, h * 65:(h + 1) * 65],
                        start=(kc == 0), stop=(kc == 15))

        otm_state = {}

        def pv_finish_a(n, g, qb):
            """Normalize block n's PV accumulators (runs at step 6):
            reciprocal on DVE, the per-token scale on ScalarE."""
            for j2 in range(2):
                pv = pv_state.pop(j2)
                rr = rrpool.tile([128, 4], F32, tag="rr", name="rr")
                pv3 = pv[:].rearrange("p (t x) -> p t x", x=65)
                nc.vector.reciprocal(rr[:], pv3[:, :, 64])
                ot = otmpool.tile([128, 256], BF16, tag=f"otm{j2}",
                                  name=f"otm{j2}")
                nc.vector.tensor_tensor(
                    ot[:].rearrange("p (t x) -> p t x", x=64),
                    pv3[:, :, 0:64],
                    rr[:].unsqueeze(-1).broadcast_to([128, 4, 64]),
                    op=mybir.AluOpType.mult)
                otm_state[j2] = ot

        def pv_finish_b(n, g, qb):
            """Transpose + store OT tiles for block n (runs at step 7)."""
            otm = {j2: otm_state.pop(j2) for j2 in range(2)}
            for ts in range(4):
                tr = ps1pool.tile([128, 128], BF16, tag="ps1", name="tr")
                for j2 in range(2):
                    nc.tensor.matmul(
                        tr[64 * j2:64 * j2 + 64, :],
                        otm[j2][:, ts * 64:(ts + 1) * 64],
                        id_sb[:], start=True, stop=True, is_transpose=True)
                nc.vector.tensor_copy(
                    OT[(g, qb)][:, ts * 128:(ts + 1) * 128], tr[:])

        # ---- output projection piece (interleaved into late blocks) ----
        wp_state = {}

        def outproj_load():
            wpcm = tc.tile_pool(name="wp", bufs=1)
            ycm = tc.tile_pool(name="ysb", bufs=2)
            wp_state["cms"] = [wpcm, ycm]
            wppool = wpcm.__enter__()
            ypool = ycm.__enter__()
            wp_all = wppool.tile([128, 4096], BF16, tag="wp_all")
            nc.sync.dma_start(
                wp_all[:].rearrange("p (j o) -> p j o", o=1024),
                wpT.rearrange("j p o -> p j o"))
            wp_state.update(wp_all=wp_all, ypool=ypool)

        def outproj_piece(tt):
            # b_proj is added on the host; this is a plain PSUM drain,
            # split across ScalarE and DVE.
            wp_all = wp_state["wp_all"]
            y_sb = wp_state["ypool"].tile([128, 1024], F32, tag="y",
                                          name="y_sb")
            for cb in range(2):
                ps = ps1pool.tile([128, 512], F32, tag="ps1", name="psy")
                for j in range(4):
                    nc.tensor.matmul(
                        ps[:],
                        OT[(j, tt // 4)][:, (tt % 4) * 128:
                                         (tt % 4 + 1) * 128],
                        wp_all[:, j * 1024 + cb * 512:
                               j * 1024 + (cb + 1) * 512],
                        start=(j == 0), stop=(j == 3))
                if cb == 0:
                    nc.scalar.copy(y_sb[:, 0:512], ps[:])
                else:
                    nc.vector.tensor_copy(y_sb[:, 512:1024], ps[:])
                nc.sync.dma_start(
                    yp[tt * 128:(tt + 1) * 128, cb * 512:(cb + 1) * 512],
                    y_sb[:, cb * 512:(cb + 1) * 512])

        # ---- emit ------------------------------------------------------
        # outproj tile tt is ready once OT[(3, tt//4)] exists, i.e. after
        # pv_finish_b(block 4*(tt//4)+3) which is emitted during block
        # 4*(tt//4)+4; schedule one tile per block starting one later so
        # PE-light blocks all carry some slack work.
        outmap = {5: [0], 6: [1], 7: [2], 8: [3], 9: [4], 10: [5],
                  11: [6], 12: [7], 13: [8], 14: [9], 15: [10, 11]}
        # startup: only the q/k tiles the first scores need; the rest of
        # projection 0, V, and projection 1 interleave into block 0 in
        # deadline order (K chunk tb feeds scores step 2*tb; V feeds
        # block 1's PV; projection 1 feeds block 1).
        qk_proj_piece(0, 0, 0)
        qk_proj_piece(0, 0, 1)
        for n, (g, qb) in enumerate(blocks):
            if n == 4:
                # x / qkv-weight tiles are dead; reuse their SBUF for the
                # output projection weights
                xbcm.__exit__(None, None, None)
                outproj_load()
            extra = []
            if n == 0:
                extra = [(qk_proj_piece, (0, tb, 1)) for tb in (1, 2, 3)]
                extra += [(v_proj_piece, (tt,)) for tt in range(8)]
                extra += [(qk_proj_piece, (1, 0, w)) for w in range(2)]
                extra += [(v_proj_piece, (tt,)) for tt in range(8, TT)]
                extra += [(qk_proj_piece, (1, tb, w))
                          for tb in (1, 2, 3) for w in range(2)]
                extra += [(qk_proj_piece, (0, tb, 0)) for tb in (1, 2, 3)]
            elif n < 3:
                extra = [(qk_proj_piece, (n + 1, tb, w))
                         for tb in range(TB) for w in range(2)]
            extra += [(outproj_piece, (tt,)) for tt in outmap.get(n, [])]
            npc = (len(extra) + 7) // 8  # extra pieces per step
            for kp in range(8):
                if n > 0:
                    if kp < 6:
                        pv_mm(n - 1, *blocks[n - 1], step=kp)
                    elif kp == 6:
                        pv_finish_a(n - 1, *blocks[n - 1])
                    else:
                        pv_finish_b(n - 1, *blocks[n - 1])
                if n == 0:
                    scores_mm(n, g, qb, kp, 0)
                    scores_mm(n, g, qb, kp, 1)
                for fn, args in extra[kp * npc:(kp + 1) * npc]:
                    fn(*args)
                if n > 0:
                    scores_mm(n, g, qb, kp, 0)
                    scores_mm(n, g, qb, kp, 1)
        for kp in range(6):
            pv_mm(15, *blocks[15], step=kp)
        pv_finish_a(15, *blocks[15])
        pv_finish_b(15, *blocks[15])
        for tt in range(12, 16):
            outproj_piece(tt)

        for cm in reversed(wp_state["cms"]):
            cm.__exit__(None, None, None)
        for cm in reversed(attncm):
            cm.__exit__(None, None, None)
        ps1cm.__exit__(None, None, None)

    nc.compile()
    return nc


_NC_CACHE = None


def get_program():
    global _NC_CACHE
    if _NC_CACHE is None:
        _NC_CACHE = build_program()
    return _NC_CACHE


def make_in_maps(x, w_qkv, b_qkv, w_proj, b_proj):
    x = np.asarray(x, dtype=np.float32)
    w_qkv = np.asarray(w_qkv, dtype=np.float32)
    b_qkv = np.asarray(b_qkv, dtype=np.float32)
    w_proj = np.asarray(w_proj, dtype=np.float32)
    b_proj = np.asarray(b_proj, dtype=np.float32)

    xTs = [np.ascontiguousarray(x[b].T).astype(BF16_NP) for b in range(B)]
    ident = np.eye(128, dtype=np.float32).astype(BF16_NP)

    grp = []
    for hg in range(2):
        sl = slice(hg * 512, (hg + 1) * 512)
        wq = w_qkv[0:C][sl]
        wk = w_qkv[C:2 * C][sl]
        wv = w_qkv[2 * C:3 * C][sl]
        bq = b_qkv[0:C][sl]
        bk = b_qkv[C:2 * C][sl]
        bv = b_qkv[2 * C:3 * C][sl]
        grp.append(dict(
            wqT=np.ascontiguousarray(wq.T).astype(BF16_NP),
            wkT=np.ascontiguousarray(wk.T).astype(BF16_NP),
            wvT=np.ascontiguousarray(wv.T).astype(BF16_NP),
            bqk=np.stack([bq[i * 128:(i + 1) * 128] for i in range(4)]
                         + [bk[i * 128:(i + 1) * 128] for i in range(4)],
                         axis=1).astype(np.float32),
            wpT=np.ascontiguousarray(
                w_proj[:, sl].T).reshape(4, 128, 1024).astype(BF16_NP),
            ident=ident,
        ))

    in_maps = []
    for core in range(N_CORES):
        b, hg = core // 2, core % 2
        m = {"xT": xTs[b]}
        m.update(grp[hg])
        in_maps.append(m)
    return in_maps


def kernel(x, w_qkv, b_qkv, w_proj, b_proj):
    nc = get_program()
    in_maps = make_in_maps(x, w_qkv, b_qkv, w_proj, b_proj)
    res = run_bass_kernel_spmd(
        nc, in_maps, core_ids=list(range(N_CORES)), trace=False)
    # V's bias is not applied on-device: softmax weights sum to 1, so it
    # contributes bv @ w_proj.T per token, folded in here with b_proj.
    bp = (np.asarray(b_proj, dtype=np.float32)
          + np.asarray(b_qkv, dtype=np.float32)[2 * C:]
          @ np.asarray(w_proj, dtype=np.float32).T)
    y = np.empty((B, T, C), dtype=np.float32)
    for b in range(B):
        y[b] = res.results[2 * b]["yp"] + res.results[2 * b + 1]["yp"] + bp
    return y


# revision 91
# speedup vs baseline: 1.0057x; 1.0052x over previous
"""Multi-head self-attention Trainium2 kernel (B=4, T=2048, C=1024, H=16, D=64).

Sharding: 8 cores = 4 batches x 2 head-groups (8 heads each). Each core
computes its batch's QKV for its heads, attention, and a partial output
projection (row-sharded over attention features). The host sums the two
partials per batch and adds b_proj + bv @ w_proj.T (the V bias times the
softmax weights, which sum to 1).

Optimizations over the straightforward version:
  - Scores run as fp8e4 DoubleRow matmuls (0.5 cycles/row): Q/K are
    quantized to fp8 by the bias-add, stored [128, 2, 2048] with the
    second k-subtile zeroed (D=64 < 128, so the pair is (dims, zeros)).
  - P@V runs transposed: out[tok, dim] with P as stationary
    ([128 keys, 128 tok] slices) and V[keys, 65] as moving (ones column
    produces the softmax denominator), so each matmul moves 65 elements
    instead of 512.
  - Softmax normalization becomes a per-partition scalar multiply, then
    PE transposes restore [feat, tok] tiles for the output projection.
  - exp is split across ScalarE (native exp) and DVE (bitwise fast-exp
    into bf16 bits); GPSIMD cannot read PSUM so only these two drain
    score tiles.
  - The whole schedule is one software pipeline: each block's score
    matmuls interleave with the previous block's P@V accumulation and
    with projection / output-projection pieces, so the PE never waits
    long on the 4-slot score-PSUM ring.
"""

import numpy as np
import ml_dtypes
from contextlib import ExitStack

import concourse.bass as bass
import concourse.bacc as bacc
import concourse.tile as tile
import concourse.mybir as mybir
from concourse.bass_utils import run_bass_kernel_spmd

F32 = mybir.dt.float32
BF16 = mybir.dt.bfloat16
FP8 = mybir.dt.float8e4
I16 = mybir.dt.int16
BF16_NP = ml_dtypes.bfloat16

B, T, C = 4, 2048, 1024
H, D = 16, 64
HL = 8          # heads per core
N_CORES = 8
CC = C // 128   # 8 contraction chunks for QKV
TB = T // 512   # 4 token blocks of 512
TT = T // 128   # 16 token chunks of 128
EXPFN = mybir.ActivationFunctionType.Exp
DR = mybir.MatmulPerfMode.DoubleRow

USE_FP8_QK = True

# Schraudolph-style exp in bf16 bit space: i16 = x*SCHR_A + SCHR_B, then
# reinterpret the int16 as bf16.  SCHR_A folds the 1/sqrt(D) score scale.
SCHR_A = 0.125 * 128.0 / float(np.log(2.0))
SCHR_B = 127.0 * 128.0 - 7.41

# exp engine per (kp, j2, kc-half) slot (32 tiles of [128,512] per
# block): 'A' = ScalarE native exp, 'V' = DVE bitwise fast-exp.  GPSIMD
# cannot read PSUM, so only these two engines can consume score tiles.
# Strict alternation keeps consecutive slots of the 4-deep PSUM ring on
# different engines; 18 A / 14 V balances the two engines' total load
# (DVE also carries reciprocals, normalize and copies).
EXP_ENG = ['A', 'V'] * 14 + ['A'] * 4


def build_program():
    nc = bacc.Bacc("TRN2", debug=False, num_devices=1, target_bir_lowering=False)

    xT = nc.dram_tensor("xT", [C, T], BF16, kind="ExternalInput").ap()
    wqT = nc.dram_tensor("wqT", [C, 512], BF16, kind="ExternalInput").ap()
    wkT = nc.dram_tensor("wkT", [C, 512], BF16, kind="ExternalInput").ap()
    wvT = nc.dram_tensor("wvT", [C, 512], BF16, kind="ExternalInput").ap()
    bqk = nc.dram_tensor("bqk", [128, 8], F32, kind="ExternalInput").ap()
    wpT = nc.dram_tensor("wpT", [4, 128, 1024], BF16, kind="ExternalInput").ap()
    ident = nc.dram_tensor("ident", [128, 128], BF16, kind="ExternalInput").ap()
    yp = nc.dram_tensor("yp", [T, C], F32, kind="ExternalOutput").ap()

    qk_dt = FP8 if USE_FP8_QK else BF16

    with tile.TileContext(nc) as tc, ExitStack() as top:
        cpool = top.enter_context(tc.tile_pool(name="consts", bufs=1))
        bqk_sb = cpool.tile([128, 8], F32, tag="bqk")
        id_sb = cpool.tile([128, 128], BF16, tag="ident")

        actpool = top.enter_context(tc.tile_pool(name="acts", bufs=1))
        # OT: attention output, feature-major [feat 128, tok 512] bf16
        OT = {(g, qb): actpool.tile([128, 512], BF16, tag=f"ot{g}_{qb}",
                                    name=f"ot{g}_{qb}")
              for g in range(4) for qb in range(4)}
        # Q/K in DoubleRow layout: [128 feat(2 heads), 2 k-subtiles, 2048 tok]
        # subtile 1 is zeros (fp8) / unused (bf16).
        QDR = {g: actpool.tile([128, 2, T], qk_dt, tag=f"qdr{g}",
                               name=f"qdr{g}") for g in range(4)}
        KDR = {g: actpool.tile([128, 2, T], qk_dt, tag=f"kdr{g}",
                               name=f"kdr{g}") for g in range(4)}
        V = [actpool.tile([128, HL * 65], BF16, tag=f"v{tt}", name=f"v{tt}")
             for tt in range(TT)]

        if USE_FP8_QK:
            # zero the second k-subtile once (matmul contracts over both)
            for g in range(4):
                nc.gpsimd.memset(QDR[g][:, 1, :], 0.0)
                nc.gpsimd.memset(KDR[g][:, 1, :], 0.0)

        ps1cm = tc.tile_pool(name="ps1", bufs=2, space="PSUM")
        ps1pool = ps1cm.__enter__()
        attncm = [tc.tile_pool(name="pt", bufs=1),
                  tc.tile_pool(name="ps2", bufs=4, space="PSUM"),
                  tc.tile_pool(name="pvtr", bufs=2, space="PSUM"),
                  tc.tile_pool(name="rr", bufs=2),
                  tc.tile_pool(name="otm", bufs=2)]
        (ptpool, ps2pool, pvpool, rrpool, otmpool) = \
            [cm.__enter__() for cm in attncm]

        xbcm = tc.tile_pool(name="xball", bufs=1)
        xbpool = xbcm.__enter__()

        # x feature-major, all of it: [128, cc, tok]; loaded in 4 big DMAs
        xar = xbpool.tile([128, CC, T], BF16, tag="xar")
        xTr = xT.rearrange("(c p) t -> p c t", p=128)
        wq_all = xbpool.tile([128, CC * 512], BF16, tag="wq_all")
        wqr = wqT.rearrange("(c p) j -> p c j", p=128)
        wqv = wq_all[:].rearrange("p (c j) -> p c j", j=512)
        wk_all = xbpool.tile([128, CC * 512], BF16, tag="wk_all")
        nc.sync.dma_start(wqv[:, 0:4], wqr[:, 0:4])
        nc.sync.dma_start(xar[:, :, 0:512], xTr[:, :, 0:512])
        nc.sync.dma_start(wqv[:, 4:8], wqr[:, 4:8])
        nc.sync.dma_start(bqk_sb[:], bqk[:])
        nc.sync.dma_start(
            wk_all[:].rearrange("p (c j) -> p c j", j=512),
            wkT.rearrange("(c p) j -> p c j", p=128))
        for tb in range(1, TB):
            nc.sync.dma_start(xar[:, :, tb * 512:(tb + 1) * 512],
                              xTr[:, :, tb * 512:(tb + 1) * 512])
        wv_all = xbpool.tile([128, CC, 512], BF16, tag="wv_all")
        nc.sync.dma_start(wv_all[:],
                          wvT.rearrange("(c p) j -> p c j", p=128))
        nc.sync.dma_start(id_sb[:], ident[:])

        def xb(tb, cc):
            return xar[:, cc, tb * 512:(tb + 1) * 512]

        def qk_proj_piece(g, tb, which):
            """One Q or K projection group for head pair g, token block tb.
            The bias add runs on ScalarE (per-partition bias AP is legal
            there) and quantizes to fp8 on the way out."""
            w_all, dst, bcol = ((wq_all, QDR[g], g) if which == 0 else
                                (wk_all, KDR[g], 4 + g))
            ps = ps1pool.tile([128, 512], F32, tag="ps1", name="psqk")
            for cc in range(CC):
                co = cc * 512 + g * 128
                nc.tensor.matmul(
                    ps[:], w_all[:, co:co + 128],
                    xb(tb, cc),
                    start=(cc == 0), stop=(cc == CC - 1))
            nc.scalar.activation(
                dst[:, 0, tb * 512:(tb + 1) * 512], ps[:],
                mybir.ActivationFunctionType.Identity,
                bias=bqk_sb[:, bcol:bcol + 1])

        def qk_proj(g):
            for tb in range(TB):
                for which in range(2):
                    qk_proj_piece(g, tb, which)

        def v_proj_piece(tt):
            ps = ps1pool.tile([128, 512], F32, tag="ps1", name="psv")
            for cc in range(CC):
                nc.tensor.matmul(
                    ps[:], xar[:, cc, tt * 128:(tt + 1) * 128],
                    wv_all[:, cc, :],
                    start=(cc == 0), stop=(cc == CC - 1))
            # V carries no bias: softmax weights sum to 1, so the bias
            # contribution is bv @ Wp per token, folded into the host-side
            # b_proj add.
            v3 = V[tt][:].rearrange("p (h x) -> p h x", x=65)
            nc.gpsimd.memset(v3[:, :, 64:65], 1.0)
            src = ps[:].rearrange("p (h x) -> p h x", x=64)
            nc.vector.tensor_copy(v3[:, :, 0:64], src)

        # ---- attention blocks, software-pipelined -----------------------
        # block n = (g, qb), qb-major so every fourth block finishes an
        # OT column and the output projection can interleave early.
        # scores(n) and PV(n-1) interleave in the PE stream so PE has
        # work while exp drains score PSUMs.
        blocks = [(g, qb) for qb in range(4) for g in range(4)]
        pts = {}      # (parity, kp, j2) -> exp'd score tile [128, 1024]
        pv_state = {}  # live PV psum tiles per j2

        def scores_mm(n, g, qb, kp, j2):
            """Scores for head 2g+j2, key chunks 2kp/2kp+1, queries qb:
            two [128, 512] tiles (one per key chunk), each exp'd as soon
            as it fills."""
            fo = 64 * j2
            for j in range(2):
                kc = 2 * kp + j
                pp = ps2pool.tile([128, 512], F32, tag="ps2",
                                  name=f"sc{j2}")
                if USE_FP8_QK:
                    for u in range(2):
                        nc.tensor.matmul(
                            pp[:, u * 256:(u + 1) * 256],
                            KDR[g][fo:fo + 64, :, kc * 128:(kc + 1) * 128],
                            QDR[g][fo:fo + 64, :,
                                   qb * 512 + u * 256:qb * 512 + (u + 1) * 256],
                            start=True, stop=True, perf_mode=DR)
                else:
                    nc.tensor.matmul(
                        pp[:],
                        KDR[g][fo:fo + 64, 0, kc * 128:(kc + 1) * 128],
                        QDR[g][fo:fo + 64, 0, qb * 512:(qb + 1) * 512],
                        start=True, stop=True)
                pt = ptpool.tile([128, 512], BF16,
                                 tag=f"pt{n % 2}_{kc}_{j2}",
                                 name=f"pt{n % 2}_{kc}_{j2}")
                if EXP_ENG[4 * kp + 2 * j2 + j] == 'A':
                    nc.scalar.activation(pt[:], pp[:], EXPFN, scale=0.125)
                else:
                    nc.vector.tensor_scalar(pt[:].bitcast(I16), pp[:],
                                            SCHR_A, SCHR_B,
                                            op0=mybir.AluOpType.mult,
                                            op1=mybir.AluOpType.add)
                pts[(n % 2, kc, j2)] = pt

        # PV accumulation order per head half: groups ts0..ts3, 16 key
        # chunks each, strictly sequential (one open accumulation group
        # per PSUM bank).  Spread over steps 0..5 so the finish work can
        # run at steps 6-7 without delaying the next block's exps.
        PV_SPLIT = [0, 11, 22, 33, 44, 54, 64]

        def pv_mm(n, g, qb, step):
            par = n % 2
            for j2 in range(2):
                h = 2 * g + j2
                if step == 0:
                    pv_state[j2] = pvpool.tile([128, 260], F32,
                                               tag="pvtr", name=f"pv{j2}")
                pv = pv_state[j2]
                for i in range(PV_SPLIT[step], PV_SPLIT[step + 1]):
                    ts, kc = i // 16, i % 16
                    nc.tensor.matmul(
                        pv[:, ts * 65:(ts + 1) * 65],
                        pts[(par, kc, j2)][:, ts * 128:(ts + 1) * 128],
                        V[kc][:, h * 65:(h + 1) * 65],
                        start=(kc == 0), stop=(kc == 15))

        otm_state = {}

        def pv_finish_a(n, g, qb):
            """Normalize block n's PV accumulators (runs at step 6):
            reciprocal on DVE, the per-token scale on ScalarE."""
            for j2 in range(2):
                pv = pv_state.pop(j2)
                rr = rrpool.tile([128, 4], F32, tag="rr", name="rr")
                pv3 = pv[:].rearrange("p (t x) -> p t x", x=65)
                nc.vector.reciprocal(rr[:], pv3[:, :, 64])
                ot = otmpool.tile([128, 256], BF16, tag=f"otm{j2}",
                                  name=f"otm{j2}")
                nc.vector.tensor_tensor(
                    ot[:].rearrange("p (t x) -> p t x", x=64),
                    pv3[:, :, 0:64],
                    rr[:].unsqueeze(-1).broadcast_to([128, 4, 64]),
                    op=mybir.AluOpType.mult)
                otm_state[j2] = ot

        def pv_finish_b(n, g, qb):
            """Transpose + store OT tiles for block n (runs at step 7)."""
            otm = {j2: otm_state.pop(j2) for j2 in range(2)}
            for ts in range(4):
                tr = ps1pool.tile([128, 128], BF16, tag="ps1", name="tr")
                for j2 in range(2):
                    nc.tensor.matmul(
                        tr[64 * j2:64 * j2 + 64, :],
                        otm[j2][:, ts * 64:(ts + 1) * 64],
                        id_sb[:], start=True, stop=True, is_transpose=True)
                nc.vector.tensor_copy(
                    OT[(g, qb)][:, ts * 128:(ts + 1) * 128], tr[:])

        # ---- output projection piece (interleaved into late blocks) ----
        wp_state = {}

        def outproj_load():
            wpcm = tc.tile_pool(name="wp", bufs=1)
            ycm = tc.tile_pool(name="ysb", bufs=2)
            wp_state["cms"] = [wpcm, ycm]
            wppool = wpcm.__enter__()
            ypool = ycm.__enter__()
            wp_all = wppool.tile([128, 4096], BF16, tag="wp_all")
            nc.sync.dma_start(
                wp_all[:].rearrange("p (j o) -> p j o", o=1024),
                wpT.rearrange("j p o -> p j o"))
            wp_state.update(wp_all=wp_all, ypool=ypool)

        def outproj_piece(tt, pool=None):
            # b_proj is added on the host; this is a plain PSUM drain,
            # split across ScalarE and DVE.  Tail pieces borrow the score
            # PSUM pool (free once the last scores have been exp'd).
            wp_all = wp_state["wp_all"]
            y_sb = wp_state["ypool"].tile([128, 1024], F32, tag="y",
                                          name="y_sb")
            for cb in range(2):
                ps = ((pool or ps1pool)
                      .tile([128, 512], F32,
                            tag="ps1" if pool is None else "ps2",
                            name="psy"))
                for j in range(4):
                    nc.tensor.matmul(
                        ps[:],
                        OT[(j, tt // 4)][:, (tt % 4) * 128:
                                         (tt % 4 + 1) * 128],
                        wp_all[:, j * 1024 + cb * 512:
                               j * 1024 + (cb + 1) * 512],
                        start=(j == 0), stop=(j == 3))
                if cb == 0:
                    nc.scalar.copy(y_sb[:, 0:512], ps[:])
                else:
                    nc.vector.tensor_copy(y_sb[:, 512:1024], ps[:])
                nc.sync.dma_start(
                    yp[tt * 128:(tt + 1) * 128, cb * 512:(cb + 1) * 512],
                    y_sb[:, cb * 512:(cb + 1) * 512])

        # ---- emit ------------------------------------------------------
        # outproj tile tt is ready once OT[(3, tt//4)] exists, i.e. after
        # pv_finish_b(block 4*(tt//4)+3) which is emitted during block
        # 4*(tt//4)+4; schedule one tile per block starting one later so
        # PE-light blocks all carry some slack work.
        outmap = {5: [0], 6: [1], 7: [2], 8: [3], 9: [4], 10: [5],
                  11: [6], 12: [7], 13: [8], 14: [9], 15: [10, 11]}
        # startup: only the q/k tiles the first scores need; the rest of
        # projection 0, V, and projection 1 interleave into block 0 in
        # deadline order (K chunk tb feeds scores step 2*tb; V feeds
        # block 1's PV; projection 1 feeds block 1).
        qk_proj_piece(0, 0, 0)
        qk_proj_piece(0, 0, 1)
        for n, (g, qb) in enumerate(blocks):
            if n == 4:
                # x / qkv-weight tiles are dead; reuse their SBUF for the
                # output projection weights
                xbcm.__exit__(None, None, None)
                outproj_load()
            extra = []
            if n == 0:
                extra = [(qk_proj_piece, (0, tb, 1)) for tb in (1, 2, 3)]
                extra += [(v_proj_piece, (tt,)) for tt in range(8)]
                extra += [(qk_proj_piece, (1, 0, w)) for w in range(2)]
                extra += [(v_proj_piece, (tt,)) for tt in range(8, TT)]
                extra += [(qk_proj_piece, (1, tb, w))
                          for tb in (1, 2, 3) for w in range(2)]
                extra += [(qk_proj_piece, (0, tb, 0)) for tb in (1, 2, 3)]
            elif n < 3:
                extra = [(qk_proj_piece, (n + 1, tb, w))
                         for tb in range(TB) for w in range(2)]
            extra += [(outproj_piece, (tt,)) for tt in outmap.get(n, [])]
            npc = (len(extra) + 7) // 8  # extra pieces per step
            for kp in range(8):
                if n > 0:
                    if kp < 6:
                        pv_mm(n - 1, *blocks[n - 1], step=kp)
                    elif kp == 6:
                        pv_finish_a(n - 1, *blocks[n - 1])
                    else:
                        pv_finish_b(n - 1, *blocks[n - 1])
                if n == 0:
                    scores_mm(n, g, qb, kp, 0)
                    scores_mm(n, g, qb, kp, 1)
                for fn, args in extra[kp * npc:(kp + 1) * npc]:
                    fn(*args)
                if n > 0:
                    scores_mm(n, g, qb, kp, 0)
                    scores_mm(n, g, qb, kp, 1)
        for kp in range(6):
            pv_mm(15, *blocks[15], step=kp)
        pv_finish_a(15, *blocks[15])
        pv_finish_b(15, *blocks[15])
        for tt in range(12, 16):
            outproj_piece(tt, pool=ps2pool)

        for cm in reversed(wp_state["cms"]):
            cm.__exit__(None, None, None)
        for cm in reversed(attncm):
            cm.__exit__(None, None, None)
        ps1cm.__exit__(None, None, None)

    nc.compile()
    return nc


_NC_CACHE = None


def get_program():
    global _NC_CACHE
    if _NC_CACHE is None:
        _NC_CACHE = build_program()
    return _NC_CACHE


def make_in_maps(x, w_qkv, b_qkv, w_proj, b_proj):
    x = np.asarray(x, dtype=np.float32)
    w_qkv = np.asarray(w_qkv, dtype=np.float32)
    b_qkv = np.asarray(b_qkv, dtype=np.float32)
    w_proj = np.asarray(w_proj, dtype=np.float32)
    b_proj = np.asarray(b_proj, dtype=np.float32)

    xTs = [np.ascontiguousarray(x[b].T).astype(BF16_NP) for b in range(B)]
    ident = np.eye(128, dtype=np.float32).astype(BF16_NP)

    grp = []
    for hg in range(2):
        sl = slice(hg * 512, (hg + 1) * 512)
        wq = w_qkv[0:C][sl]
        wk = w_qkv[C:2 * C][sl]
        wv = w_qkv[2 * C:3 * C][sl]
        bq = b_qkv[0:C][sl]
        bk = b_qkv[C:2 * C][sl]
        bv = b_qkv[2 * C:3 * C][sl]
        grp.append(dict(
            wqT=np.ascontiguousarray(wq.T).astype(BF16_NP),
            wkT=np.ascontiguousarray(wk.T).astype(BF16_NP),
            wvT=np.ascontiguousarray(wv.T).astype(BF16_NP),
            bqk=np.stack([bq[i * 128:(i + 1) * 128] for i in range(4)]
                         + [bk[i * 128:(i + 1) * 128] for i in range(4)],
                         axis=1).astype(np.float32),
            wpT=np.ascontiguousarray(
                w_proj[:, sl].T).reshape(4, 128, 1024).astype(BF16_NP),
            ident=ident,
        ))

    in_maps = []
    for core in range(N_CORES):
        b, hg = core // 2, core % 2
        m = {"xT": xTs[b]}
        m.update(grp[hg])
        in_maps.append(m)
    return in_maps


def kernel(x, w_qkv, b_qkv, w_proj, b_proj):
    nc = get_program()
    in_maps = make_in_maps(x, w_qkv, b_qkv, w_proj, b_proj)
    res = run_bass_kernel_spmd(
        nc, in_maps, core_ids=list(range(N_CORES)), trace=False)
    # V's bias is not applied on-device: softmax weights sum to 1, so it
    # contributes bv @ w_proj.T per token, folded in here with b_proj.
    bp = (np.asarray(b_proj, dtype=np.float32)
          + np.asarray(b_qkv, dtype=np.float32)[2 * C:]
          @ np.asarray(w_proj, dtype=np.float32).T)
    y = np.empty((B, T, C), dtype=np.float32)
    for b in range(B):
        y[b] = res.results[2 * b]["yp"] + res.results[2 * b + 1]["yp"] + bp
    return y
